# revision 1
# baseline (speedup 1.0000x reference)
"""Trainium2 Bass kernel for ConvChebTemp (Chebyshev graph conv with temporal weights).

Math: out[b,v,o] = sum_{k,t,f} T_k(L)x0[:,t,f,b] w[f,k,t,o] + bias[o]
with x0 = inputs permuted to [V, T*Fin*B] and T_k the Chebyshev recurrence.

Key reformulation (Clenshaw): since the f/t contraction commutes with L,
contract weights FIRST: z_k[v, b, o] = sum_{t,f} x0[v,t,f,b] w[f,k,t,o], then
  b3 = z3; b2 = z2 + 2 L b3; b1 = z1 + 2 L b2 - b3; out = z0 + L b1 - b2 + bias
This shrinks every SpMM's column count 4x (256 -> 64 per batch).

Sharding: data-parallel over batch B=16 -> 2 batches per core, 8 cores.
Each SpMM is gather (dma_gather, sorted-by-row CSR, 512 B rows) + per-chunk
selection matmuls on the PE (selection matrices built on-chip in one DVE
tensor_scalar op from O(NNZ) metadata).
"""
import sys

sys.path.insert(0, "/opt/trn_rl_repo")

from contextlib import ExitStack  # noqa: E402

import numpy as np  # noqa: E402

from concourse import bacc, bass, mybir, tile  # noqa: E402
from concourse.bass_utils import run_bass_kernel_spmd  # noqa: E402

P = 128
N_CORES = 8
FP32 = mybir.dt.float32
I16 = mybir.dt.int16

# Problem dims (hardcoded per spec)
B, V, T, FIN = 16, 12288, 4, 64
KV, KT, FOUT = 4, 4, 64
BC = B // N_CORES          # batches per core
F = BC * FOUT              # spmm column width per core (both batches interleaved)
C = T * FIN                # z-matmul contraction dim
GBUF_BUFS = 6
PSZ_BUFS = 2
PST_BUFS = 2
PSS_BUFS = 4


def _preprocess_lap(lap_rows, lap_cols, lap_vals, v):
    """Sort nnz by row, pad each 128-row out-tile's run to a multiple of P.

    Returns (gidx [16, NNZP//16] int16 wrapped, growl [P, NCHUNK] f32,
    gval [P, NCHUNK] f32, chunks_per_tile list).
    """
    nt = v // P
    order = np.argsort(lap_rows, kind="stable")
    srows = lap_rows[order]
    scols = lap_cols[order]
    svals = lap_vals[order]
    tile_of = srows // P
    # counts per tile
    counts = np.bincount(tile_of, minlength=nt)
    chunks_per_tile = [max(1, int(-(-c // P))) for c in counts]
    nnzp = sum(chunks_per_tile) * P
    gidx = np.zeros(nnzp, np.int16)
    growl = np.zeros(nnzp, np.float32)
    gval = np.zeros(nnzp, np.float32)
    # fill per tile
    starts = np.zeros(nt + 1, np.int64)
    np.cumsum(counts, out=starts[1:])
    pos = 0
    for t in range(nt):
        n = int(counts[t])
        s = int(starts[t])
        gidx[pos:pos + n] = scols[s:s + n]
        growl[pos:pos + n] = (srows[s:s + n] - t * P).astype(np.float32)
        gval[pos:pos + n] = svals[s:s + n]
        # padding slots: col 0, rowl 0, val 0 (contribute nothing)
        pos += chunks_per_tile[t] * P
    assert pos == nnzp
    nchunk = nnzp // P
    # wrapped int16 layout for dma_gather: slot s -> [s % 16, s // 16]
    gidx_w = gidx.reshape(-1, 16).T.copy()          # [16, NNZP//16]
    gidx_w = np.tile(gidx_w, (8, 1))                # replicate for 8 q7 cores
    growl_m = growl.reshape(nchunk, P).T.copy()     # [P, NCHUNK]
    gval_m = gval.reshape(nchunk, P).T.copy()       # [P, NCHUNK]
    return gidx_w, growl_m, gval_m, chunks_per_tile


def build_program(v, chunks_per_tile, n_cores=N_CORES, max_phase=3):
    """Build the SPMD Bass program (identical across cores)."""
    nt = v // P
    nchunk = sum(chunks_per_tile)
    nnzp = nchunk * P
    nc = bacc.Bacc("TRN2", target_bir_lowering=False, debug=False,
                   num_devices=n_cores)

    xin = nc.dram_tensor("xin", [BC, v, T, FIN], FP32, kind="ExternalInput")
    wz = nc.dram_tensor("wz", [P, 2 * KV * FOUT], FP32, kind="ExternalInput")
    bias_d = nc.dram_tensor("bias128", [P, F], FP32, kind="ExternalInput")
    iota_d = nc.dram_tensor("iota128", [P, P], FP32, kind="ExternalInput")
    ident_d = nc.dram_tensor("ident128", [P, P], FP32, kind="ExternalInput")
    gidx_d = nc.dram_tensor("gidx", [P, nnzp // 16], I16, kind="ExternalInput")
    growl_d = nc.dram_tensor("growl", [P, nchunk], FP32, kind="ExternalInput")
    gval1_d = nc.dram_tensor("gval1", [P, nchunk], FP32, kind="ExternalInput")
    gval2_d = nc.dram_tensor("gval2", [P, nchunk], FP32, kind="ExternalInput")
    out_d = nc.dram_tensor("out", [BC, v, FOUT], FP32, kind="ExternalOutput")

    with tile.TileContext(nc) as tc, ExitStack() as ctx:
        dram = ctx.enter_context(tc.tile_pool(name="dram", bufs=1, space="DRAM"))
        z0_d = dram.tile([v, F], FP32, tag="z0d")
        b3_d = dram.tile([v, F], FP32, tag="b3d")
        b2_d = dram.tile([v, F], FP32, tag="b2d")
        b1_d = dram.tile([v, F], FP32, tag="b1d")

        const = ctx.enter_context(tc.tile_pool(name="const", bufs=1))
        res = ctx.enter_context(tc.tile_pool(name="res", bufs=1))
        xpool = ctx.enter_context(tc.tile_pool(name="x", bufs=3))
        xtp = ctx.enter_context(tc.tile_pool(name="xt", bufs=3))
        stg = ctx.enter_context(tc.tile_pool(name="stg", bufs=3))
        gpool = ctx.enter_context(tc.tile_pool(name="gbuf", bufs=GBUF_BUFS))
        spool = ctx.enter_context(tc.tile_pool(name="sel", bufs=4))
        bpool = ctx.enter_context(tc.tile_pool(name="bt", bufs=3))
        tpool = ctx.enter_context(tc.tile_pool(name="tmp", bufs=3))
        psz = ctx.enter_context(tc.tile_pool(name="psz", bufs=PSZ_BUFS, space="PSUM"))
        pst = ctx.enter_context(tc.tile_pool(name="pst", bufs=PST_BUFS, space="PSUM"))
        pss = ctx.enter_context(tc.tile_pool(name="pss", bufs=PSS_BUFS, space="PSUM"))

        # constants + metadata resident in SBUF
        iota_sb = const.tile([P, P], FP32, tag="iota")
        nc.sync.dma_start(iota_sb[:], iota_d[:, :])
        ident_sb = const.tile([P, P], FP32, tag="ident")
        nc.sync.dma_start(ident_sb[:], ident_d[:, :])
        bias_sb = const.tile([P, F], FP32, tag="bias")
        nc.sync.dma_start(bias_sb[:], bias_d[:, :])
        wz_sb = const.tile([P, 2 * KV * FOUT], FP32, tag="wz")
        nc.sync.dma_start(wz_sb[:], wz[:, :])
        gidx_sb = const.tile([P, nnzp // 16], I16, tag="gidx")
        nc.sync.dma_start(gidx_sb[:], gidx_d[:, :])
        growl_sb = const.tile([P, nchunk], FP32, tag="growl")
        nc.sync.dma_start(growl_sb[:], growl_d[:, :])
        gval1_sb = const.tile([P, nchunk], FP32, tag="gval1")
        nc.sync.dma_start(gval1_sb[:], gval1_d[:, :])
        gval2_sb = const.tile([P, nchunk], FP32, tag="gval2")
        nc.sync.dma_start(gval2_sb[:], gval2_d[:, :])

        # per-vt 256-col block: [z1_b0 | z2_b0 | z1_b1 | z2_b1]
        z12_res = res.tile([P, nt * 2 * F], FP32, tag="z12")
        z12v = z12_res[:].rearrange("p (t x o) -> p t x o", x=4, o=FOUT)

        # ---------- phase Z: z_k = x0 @ w_k for all k ----------
        for vt in range(nt):
            v0 = vt * P
            # stage layout: [z0_b0 | z3_b0 | z0_b1 | z3_b1]
            st = stg.tile([P, 2 * F], FP32, tag="st")
            stv = st[:].rearrange("p (x o) -> p x o", o=FOUT)
            for b in range(BC):
                xt = xpool.tile([P, C], FP32, tag="xnat")
                nc.sync.dma_start(
                    xt[:], xin[b, v0:v0 + P, :, :].rearrange("p t f -> p (t f)"))
                tps = pst.tile([P, C], FP32, tag="tps")
                for cc in range(2):
                    nc.tensor.matmul(tps[:, cc * P:(cc + 1) * P],
                                     lhsT=xt[:, cc * P:(cc + 1) * P],
                                     rhs=ident_sb[:], is_transpose=True,
                                     start=True, stop=True)
                xT2 = xtp.tile([P, C], FP32, tag="xT")
                nc.vector.tensor_copy(xT2[:], tps[:])
                zps = psz.tile([P, KV * FOUT], FP32, tag="zps")
                for cc in range(2):
                    nc.tensor.matmul(zps[:], lhsT=xT2[:, cc * P:(cc + 1) * P],
                                     rhs=wz_sb[:, cc * KV * FOUT:(cc + 1) * KV * FOUT],
                                     start=(cc == 0), stop=(cc == 1))
                # zps cols = [z0 | z3 | z1 | z2] for this b
                nc.vector.tensor_copy(st[:, b * F:(b + 1) * F], zps[:, 0:F])
                nc.vector.tensor_copy(z12_res[:, vt * 2 * F + b * F:
                                              vt * 2 * F + (b + 1) * F],
                                      zps[:, F:2 * F])
            nc.sync.dma_start(
                z0_d[v0:v0 + P, :].rearrange("p (x o) -> p x o", o=FOUT),
                stv[:, 0::2, :])
            nc.sync.dma_start(
                b3_d[v0:v0 + P, :].rearrange("p (x o) -> p x o", o=FOUT),
                stv[:, 1::2, :])

        # ---------- spmm phases ----------
        # dma_gather is capped at 1024 indices per instruction (the SWDGE
        # descriptor ring holds 16 rings x 64 descs); gather in 8-chunk pieces
        # that may span out-tile boundaries.
        CHUNKS_PER_PIECE = 8

        def spmm_phase(src_d, vals_sb, combine):
            state = {"gb": None, "base": 0, "len": 0}

            def ensure_piece(c):
                while state["gb"] is None or c >= state["base"] + state["len"]:
                    base = 0 if state["gb"] is None else state["base"] + state["len"]
                    plen = min(CHUNKS_PER_PIECE, nchunk - base)
                    gb = gpool.tile([P, plen, P], FP32, tag="gb")
                    s0 = base * P
                    nidx = plen * P
                    nc.gpsimd.dma_gather(
                        out_ap=gb[:],
                        in_ap=src_d[:, :],
                        idxs_ap=gidx_sb[:, s0 // 16:(s0 + nidx) // 16],
                        num_idxs=nidx,
                        num_idxs_reg=nidx,
                        elem_size=F,
                    )
                    state.update(gb=gb, base=base, len=plen)
                return state["gb"], state["base"]

            ci = 0
            for tt in range(nt):
                nck = chunks_per_tile[tt]
                ps = pss.tile([P, F], FP32, tag="ps")
                for k in range(nck):
                    col = ci + k
                    gb, base = ensure_piece(col)
                    sT = spool.tile([P, P], FP32, tag="sT")
                    nc.vector.tensor_scalar(
                        out=sT[:], in0=iota_sb[:],
                        scalar1=growl_sb[:, col:col + 1],
                        scalar2=vals_sb[:, col:col + 1],
                        op0=mybir.AluOpType.is_equal,
                        op1=mybir.AluOpType.mult,
                    )
                    nc.tensor.matmul(ps[:], lhsT=sT[:], rhs=gb[:, col - base, :],
                                     start=(k == 0), stop=(k == nck - 1))
                combine(tt, ps)
                ci += nck

        def ps3(ps):
            return ps[:].rearrange("p (x o) -> p x o", o=FOUT)

        def dram3(d, tt):
            return d[tt * P:(tt + 1) * P, :].rearrange("p (x o) -> p x o", o=FOUT)

        # spmm 1: b2 = z2 + 2 L b3   (z2 slots become b2 in place)
        def combine1(tt, ps):
            zsl = z12v[:, tt, 1::2, :]
            nc.vector.tensor_tensor(out=zsl, in0=ps3(ps), in1=zsl,
                                    op=mybir.AluOpType.add)
            nc.sync.dma_start(dram3(b2_d, tt), zsl)

        if max_phase >= 1:
            spmm_phase(b3_d, gval2_sb, combine1)

        # spmm 2: b1 = z1 + 2 L b2 - b3
        def combine2(tt, ps):
            zsl = z12v[:, tt, 0::2, :]
            b3t = bpool.tile([P, F], FP32, tag="b3t")
            nc.sync.dma_start(b3t[:], b3_d[tt * P:(tt + 1) * P, :])
            tmp = tpool.tile([P, F], FP32, tag="tmp")
            nc.vector.tensor_tensor(out=ps3(tmp), in0=ps3(ps), in1=zsl,
                                    op=mybir.AluOpType.add)
            nc.vector.tensor_tensor(out=tmp[:], in0=tmp[:], in1=b3t[:],
                                    op=mybir.AluOpType.subtract)
            nc.sync.dma_start(b1_d[tt * P:(tt + 1) * P, :], tmp[:])

        if max_phase >= 2:
            spmm_phase(b2_d, gval2_sb, combine2)

        # spmm 3: out = z0 + L b1 - b2 + bias
        def combine3(tt, ps):
            b2sl = z12v[:, tt, 1::2, :]
            z0t = bpool.tile([P, F], FP32, tag="z0t")
            nc.sync.dma_start(z0t[:], z0_d[tt * P:(tt + 1) * P, :])
            tmp = tpool.tile([P, F], FP32, tag="otmp")
            nc.vector.tensor_tensor(out=ps3(tmp), in0=ps3(ps), in1=b2sl,
                                    op=mybir.AluOpType.subtract)
            nc.vector.tensor_tensor(out=tmp[:], in0=tmp[:], in1=z0t[:],
                                    op=mybir.AluOpType.add)
            nc.vector.tensor_tensor(out=tmp[:], in0=tmp[:], in1=bias_sb[:],
                                    op=mybir.AluOpType.add)
            for b in range(BC):
                nc.sync.dma_start(out_d[b, tt * P:(tt + 1) * P, :],
                                  tmp[:, b * FOUT:(b + 1) * FOUT])

        if max_phase >= 3:
            spmm_phase(b1_d, gval1_sb, combine3)

    nc.compile()
    return nc


def make_host_inputs(inputs, weight, bias, lap_vals, lap_rows, lap_cols, v=V):
    """Build the per-core input maps + preprocessing. Returns (in_maps, chunks)."""
    gidx_w, growl_m, gval_m, chunks = _preprocess_lap(
        np.asarray(lap_rows), np.asarray(lap_cols),
        np.asarray(lap_vals, np.float32), v)
    w = np.asarray(weight, np.float32)
    # wz[cc, c_local, k*FOUT+o] where c = t*FIN+f = cc*128+c_local
    w = w[:, [0, 3, 1, 2], :, :]  # k order [z0, z3, z1, z2]
    wz = np.transpose(w, (2, 0, 1, 3)).reshape(C, KV * FOUT)  # [(t f), (k o)]
    # [c, ko] -> [c_local, cc*256 + ko]
    wz = np.ascontiguousarray(
        wz.reshape(2, P, KV * FOUT).transpose(1, 0, 2).reshape(P, 2 * KV * FOUT))
    bias128 = np.ascontiguousarray(
        np.tile(np.asarray(bias, np.float32), (P, BC)))
    iota128 = np.ascontiguousarray(
        np.broadcast_to(np.arange(P, dtype=np.float32)[None, :], (P, P)))
    ident128 = np.eye(P, dtype=np.float32)
    common = {
        "wz": wz,
        "bias128": bias128,
        "iota128": iota128,
        "ident128": ident128,
        "gidx": np.ascontiguousarray(gidx_w),
        "growl": np.ascontiguousarray(growl_m),
        "gval1": np.ascontiguousarray(gval_m),
        "gval2": np.ascontiguousarray(2.0 * gval_m),
    }
    xin = np.asarray(inputs, np.float32)
    in_maps = []
    for r in range(N_CORES):
        m = dict(common)
        m["xin"] = np.ascontiguousarray(xin[BC * r:BC * (r + 1)])
        in_maps.append(m)
    return in_maps, chunks


_CACHE = {}


def _get_program(chunks):
    key = tuple(chunks)
    if key not in _CACHE:
        _CACHE[key] = build_program(V, list(chunks))
    return _CACHE[key]


def kernel(inputs, weight, bias, lap_vals, lap_rows, lap_cols):
    in_maps, chunks = make_host_inputs(inputs, weight, bias, lap_vals,
                                       lap_rows, lap_cols)
    nc = _get_program(chunks)
    res = run_bass_kernel_spmd(nc, in_maps, list(range(N_CORES)))
    out = np.concatenate([res.results[r]["out"] for r in range(N_CORES)], axis=0)
    return np.ascontiguousarray(out.astype(np.float32))


def time_kernel(inputs_dict, iters=3):
    """Wall-clock repeated executions of the cached program (ns per run)."""
    import time

    in_maps, chunks = make_host_inputs(**inputs_dict)
    nc = _get_program(chunks)
    times = []
    for _ in range(iters):
        t0 = time.perf_counter()
        run_bass_kernel_spmd(nc, in_maps, list(range(N_CORES)))
        times.append(time.perf_counter() - t0)
    return min(times) * 1e9



# revision 5
# speedup vs baseline: 1.9236x; 1.9236x over previous
"""Trainium2 Bass kernel for ConvChebTemp (Chebyshev graph conv, temporal weights).

Math: out[b,v,o] = sum_{k,t,f} T_k(L)x0[:,t,f,b] w[f,k,t,o] + bias[o]
with x0 = inputs permuted to [V, T*Fin*B] and T_k the Chebyshev recurrence.

Clenshaw reformulation (weights contracted first):
  z_k[v,b,o] = sum_{t,f} x0[v,t,f,b] w[f,k,t,o]
  b3 = z3; b2 = z2 + 2 L b3; b1 = z1 + 2 L b2 - b3; out = z0 + L b1 - b2 + bias

Sharding: 8 cores = 4 pairs. Pair p owns batches [4p, 4p+4); within the pair
the graph rows are split in half (core 2p: rows [0, V/2), core 2p+1 the rest).
The Clenshaw iterates b3/b2/b1 live in pair-SHARED HBM tensors
(addr_space="Shared": cores (2k, 2k+1) see one physical buffer), so each
core writes only its half and gathers from the full tensor. Cross-core
ordering is a tiny per-pair AllGather barrier before each phase's gathers.

Everything on the SpMM path is bf16: gather rows are 4 batches x 64 Fout x 2B
= 512B (full DMA descriptor efficiency) and all matmuls run at 1 cycle/row.
"""
import sys

sys.path.insert(0, "/opt/trn_rl_repo")

from contextlib import ExitStack  # noqa: E402

import ml_dtypes  # noqa: E402
import numpy as np  # noqa: E402

from concourse import bacc, bass, mybir, tile  # noqa: E402
from concourse.bass_utils import run_bass_kernel_spmd  # noqa: E402

P = 128
N_CORES = 8
FP32 = mybir.dt.float32
BF16 = mybir.dt.bfloat16
I32 = mybir.dt.int32
I16 = mybir.dt.int16

# Problem dims (hardcoded per spec)
B, V, T, FIN = 16, 12288, 4, 64
KV, KT, FOUT = 4, 4, 64
VH = V // 2                # rows per core
NT = VH // P               # out-tiles per core (48)
BG = 4                     # batches per pair
F = BG * FOUT              # spmm row width (256 bf16 = 512B)
C = T * FIN                # z contraction dim (256)
PAIR_GROUPS = [[0, 1], [2, 3], [4, 5], [6, 7]]
CHUNKS_PER_PIECE = 8       # 1024 gather indices per instruction (SWDGE ring cap)
WGRP = 8                   # out-tiles per batched shared-HBM write


def _preprocess_lap(lap_rows, lap_cols, lap_vals):
    """Split nnz by row-half, sort by local row, pad to a common per-tile
    chunk structure (identical across cores so one SPMD program serves all).

    Returns (per_parity list of (gidx_wrapped, growl, gval), chunks_per_tile).
    """
    halves = []
    counts_h = []
    for h in (0, 1):
        lo, hi = h * VH, (h + 1) * VH
        m = (lap_rows >= lo) & (lap_rows < hi)
        lrows = lap_rows[m] - lo
        order = np.argsort(lrows, kind="stable")
        lrows = lrows[order]
        cols = lap_cols[m][order]
        vals = lap_vals[m][order]
        counts = np.bincount(lrows // P, minlength=NT)
        halves.append((lrows, cols, vals, counts))
        counts_h.append(counts)
    chunks_per_tile = [
        max(1, int(-(-counts_h[0][t] // P)), int(-(-counts_h[1][t] // P)))
        for t in range(NT)
    ]
    nchunk = sum(chunks_per_tile)
    nnzp = nchunk * P
    out = []
    for lrows, cols, vals, counts in halves:
        gidx = np.zeros(nnzp, np.int16)
        growl = np.zeros(nnzp, np.float32)
        gval = np.zeros(nnzp, np.float32)
        starts = np.zeros(NT + 1, np.int64)
        np.cumsum(counts, out=starts[1:])
        pos = 0
        for t in range(NT):
            n = int(counts[t])
            s = int(starts[t])
            gidx[pos:pos + n] = cols[s:s + n]
            growl[pos:pos + n] = (lrows[s:s + n] - t * P).astype(np.float32)
            gval[pos:pos + n] = vals[s:s + n]
            pos += chunks_per_tile[t] * P
        assert pos == nnzp
        gidx_w = np.tile(gidx.reshape(-1, 16).T.copy(), (8, 1))  # [128, nnzp/16]
        growl_m = growl.reshape(nchunk, P).T.copy()
        gval_m = gval.reshape(nchunk, P).T.copy()
        out.append((np.ascontiguousarray(gidx_w),
                    np.ascontiguousarray(growl_m),
                    np.ascontiguousarray(gval_m)))
    return out, chunks_per_tile


def build_program(chunks_per_tile, n_cores=N_CORES):
    nt = NT
    nchunk = sum(chunks_per_tile)
    nnzp = nchunk * P
    nc = bacc.Bacc("TRN2", target_bir_lowering=False, debug=False,
                   num_devices=n_cores)

    xt_d = nc.dram_tensor("xt", [BG, 2, P, VH], BF16, kind="ExternalInput")
    wz_d = nc.dram_tensor("wz", [P, 2, KV * FOUT], BF16, kind="ExternalInput")
    bias_d = nc.dram_tensor("bias128", [P, F], FP32, kind="ExternalInput")
    iota_d = nc.dram_tensor("iota128", [P, P], FP32, kind="ExternalInput")
    offt_d = nc.dram_tensor("offt", [1, 1], I32, kind="ExternalInput")
    gidx_d = nc.dram_tensor("gidx", [P, nnzp // 16], I16, kind="ExternalInput")
    growl_d = nc.dram_tensor("growl", [P, nchunk], FP32, kind="ExternalInput")
    gval1_d = nc.dram_tensor("gval1", [P, nchunk], FP32, kind="ExternalInput")
    gval2_d = nc.dram_tensor("gval2", [P, nchunk], FP32, kind="ExternalInput")
    out_d = nc.dram_tensor("out", [VH, F], FP32, kind="ExternalOutput")

    # pair-shared Clenshaw iterates (both cores of a pair see one buffer)
    bsh = [nc.dram_tensor(f"bsh{k}", [V, F], BF16, kind="Internal",
                          addr_space="Shared") for k in range(3)]
    bin_d = [nc.dram_tensor(f"bin{k}", [1, 16], BF16, kind="Internal")
             for k in range(3)]
    bout_d = [nc.dram_tensor(f"bout{k}", [2, 16], BF16, kind="Internal")
              for k in range(3)]

    with tile.TileContext(nc) as tc, ExitStack() as ctx:
        const = ctx.enter_context(tc.tile_pool(name="const", bufs=1))
        zres = ctx.enter_context(tc.tile_pool(name="zres", bufs=1))
        xpool = ctx.enter_context(tc.tile_pool(name="x", bufs=2))
        gpool = ctx.enter_context(tc.tile_pool(name="gbuf", bufs=6))
        spool = ctx.enter_context(tc.tile_pool(name="sel", bufs=4))
        opool = ctx.enter_context(tc.tile_pool(name="ostg", bufs=3))
        bpool = ctx.enter_context(tc.tile_pool(name="bounce", bufs=1))
        psz = ctx.enter_context(tc.tile_pool(name="psz", bufs=3, space="PSUM"))
        pss = ctx.enter_context(tc.tile_pool(name="pss", bufs=4, space="PSUM"))

        # constants + metadata resident in SBUF
        iota_sb = const.tile([P, P], FP32, tag="iota")
        nc.sync.dma_start(iota_sb[:], iota_d[:, :])
        bias_sb = const.tile([P, F], FP32, tag="bias")
        nc.sync.dma_start(bias_sb[:], bias_d[:, :])
        wz_sb = const.tile([P, 2, KV * FOUT], BF16, tag="wz")
        nc.sync.dma_start(wz_sb[:], wz_d[:, :, :])
        gidx_sb = const.tile([P, nnzp // 16], I16, tag="gidx")
        nc.sync.dma_start(gidx_sb[:], gidx_d[:, :])
        growl_sb = const.tile([P, nchunk], FP32, tag="growl")
        nc.sync.dma_start(growl_sb[:], growl_d[:, :])
        gval1_sb = const.tile([P, nchunk], FP32, tag="gval1")
        nc.sync.dma_start(gval1_sb[:], gval1_d[:, :])
        gval2_sb = const.tile([P, nchunk], FP32, tag="gval2")
        nc.sync.dma_start(gval2_sb[:], gval2_d[:, :])

        # my row offset into the shared [V, F] tensors (0 or VH)
        off_reg = nc.sync.alloc_register("slab_off")
        nc.sync.reg_load(off_reg, offt_d[0:1, 0:1])
        off = nc.sync.snap(off_reg, donate=True, min_val=0, max_val=VH)

        # all z_k resident in SBUF: [P, nt, KV, BG, FOUT] bf16 (96KB/partition)
        z_sb = zres.tile([P, nt, KV, BG, FOUT], BF16, tag="z")

        # ---------- phase Z: z_k = x0 @ w_k ----------
        for b in range(BG):
            xb = xpool.tile([P, 2, VH], BF16, tag="xb")
            nc.sync.dma_start(xb[:], xt_d[b, :, :, :].rearrange("c p v -> p c v"))
            for vt in range(nt):
                zps = psz.tile([P, KV * FOUT], FP32, tag="zps")
                for cc in range(2):
                    nc.tensor.matmul(zps[:],
                                     lhsT=xb[:, cc, vt * P:(vt + 1) * P],
                                     rhs=wz_sb[:, cc, :],
                                     start=(cc == 0), stop=(cc == 1))
                nc.vector.tensor_copy(
                    z_sb[:, vt, :, b, :],
                    zps[:].rearrange("p (k o) -> p k o", o=FOUT))

        shared_writes = {0: [], 1: [], 2: []}

        def write_half(kidx, kslot, grp):
            """Batched write of WGRP tiles of z-slot kslot to shared bsh[kidx]."""
            g0 = grp * WGRP
            dst = bsh[kidx][bass.ds(off + g0 * P, WGRP * P), :] \
                .rearrange("(t p) f -> p t f", p=P)
            src = z_sb[:, g0:g0 + WGRP, kslot, :, :] \
                .rearrange("p t b o -> p t (b o)")
            w = nc.sync.dma_start(dst, src)
            shared_writes[kidx].append(w)

        for grp in range(nt // WGRP):
            write_half(0, 3, grp)

        def pair_barrier(k):
            bsb = bpool.tile([1, 16], BF16, tag=f"bsb{k}")
            rd = nc.sync.dma_start(bsb[:], bsh[k][0:1, 0:16])
            # the bounce read must follow ALL my writes to bsh[k], not just
            # the group that happens to overlap row 0
            for w in shared_writes[k]:
                bass._add_dep_helper(rd.ins, w.ins, sync=True,
                                     reason="barrier after all shared writes")
            nc.sync.dma_start(bin_d[k][0:1, :], bsb[:])
            return nc.gpsimd.collective_compute(
                "AllGather", mybir.AluOpType.bypass, PAIR_GROUPS,
                ins=[bin_d[k][0:1, :]], outs=[bout_d[k][:, :]])

        # ---------- spmm phases ----------
        def spmm_phase(src_d, vals_sb, cc_inst, combine):
            state = {"gb": None, "base": 0, "len": 0}

            def ensure_piece(c):
                while state["gb"] is None or c >= state["base"] + state["len"]:
                    base = 0 if state["gb"] is None else state["base"] + state["len"]
                    plen = min(CHUNKS_PER_PIECE, nchunk - base)
                    gb = gpool.tile([P, plen, F], BF16, tag="gb")
                    s0 = base * P
                    nidx = plen * P
                    g = nc.gpsimd.dma_gather(
                        out_ap=gb[:],
                        in_ap=src_d[:, :],
                        idxs_ap=gidx_sb[:, s0 // 16:(s0 + nidx) // 16],
                        num_idxs=nidx,
                        num_idxs_reg=nidx,
                        elem_size=F,
                    )
                    bass._add_dep_helper(g.ins, cc_inst.ins, sync=True,
                                         reason="pair barrier before gather")
                    state.update(gb=gb, base=base, len=plen)
                return state["gb"], state["base"]

            ci = 0
            for tt in range(nt):
                nck = chunks_per_tile[tt]
                ps = pss.tile([P, F], FP32, tag="ps")
                for k in range(nck):
                    col = ci + k
                    gb, base = ensure_piece(col)
                    sT = spool.tile([P, P], BF16, tag="sT")
                    nc.vector.tensor_scalar(
                        out=sT[:], in0=iota_sb[:],
                        scalar1=growl_sb[:, col:col + 1],
                        scalar2=vals_sb[:, col:col + 1],
                        op0=mybir.AluOpType.is_equal,
                        op1=mybir.AluOpType.mult,
                    )
                    nc.tensor.matmul(ps[:], lhsT=sT[:], rhs=gb[:, col - base, :],
                                     start=(k == 0), stop=(k == nck - 1))
                combine(tt, ps)
                ci += nck

        def zslot(vt, k):
            return z_sb[:, vt, k, :, :].rearrange("p b o -> p (b o)")

        def ps3(ps):
            return ps[:].rearrange("p (b o) -> p b o", o=FOUT)

        # phase 1: b2 = z2 + 2 L b3   (result overwrites z2 slot)
        cc0 = pair_barrier(0)

        def combine1(tt, ps):
            nc.vector.tensor_tensor(out=zslot(tt, 2), in0=zslot(tt, 2),
                                    in1=ps[:], op=mybir.AluOpType.add)
            if (tt + 1) % WGRP == 0:
                write_half(1, 2, tt // WGRP)

        spmm_phase(bsh[0], gval2_sb, cc0, combine1)

        # phase 2: b1 = z1 + 2 L b2 - b3   (result overwrites z1 slot)
        cc1 = pair_barrier(1)

        def combine2(tt, ps):
            nc.vector.tensor_tensor(out=zslot(tt, 1), in0=zslot(tt, 1),
                                    in1=ps[:], op=mybir.AluOpType.add)
            nc.vector.tensor_tensor(out=zslot(tt, 1), in0=zslot(tt, 1),
                                    in1=zslot(tt, 3), op=mybir.AluOpType.subtract)
            if (tt + 1) % WGRP == 0:
                write_half(2, 1, tt // WGRP)

        spmm_phase(bsh[1], gval2_sb, cc1, combine2)

        # phase 3: out = z0 + L b1 - b2 + bias
        cc2 = pair_barrier(2)

        def combine3(tt, ps):
            ot = opool.tile([P, F], FP32, tag="ot")
            nc.vector.tensor_tensor(out=ot[:], in0=ps[:], in1=zslot(tt, 2),
                                    op=mybir.AluOpType.subtract)
            nc.vector.tensor_tensor(out=ot[:], in0=ot[:], in1=zslot(tt, 0),
                                    op=mybir.AluOpType.add)
            nc.vector.tensor_tensor(out=ot[:], in0=ot[:], in1=bias_sb[:],
                                    op=mybir.AluOpType.add)
            nc.sync.dma_start(out_d[tt * P:(tt + 1) * P, :], ot[:])

        spmm_phase(bsh[2], gval1_sb, cc2, combine3)

    nc.compile()
    return nc


def make_host_inputs(inputs, weight, bias, lap_vals, lap_rows, lap_cols):
    per_parity, chunks = _preprocess_lap(
        np.asarray(lap_rows), np.asarray(lap_cols),
        np.asarray(lap_vals, np.float32))
    w = np.asarray(weight, np.float32)
    # wz[(t,f) split cc, (k,o)]
    wz = np.transpose(w, (2, 0, 1, 3)).reshape(C, KV * FOUT)
    wz = np.ascontiguousarray(
        wz.reshape(2, P, KV * FOUT).transpose(1, 0, 2)).astype(ml_dtypes.bfloat16)
    bias128 = np.ascontiguousarray(
        np.tile(np.asarray(bias, np.float32), (P, BG)))
    iota128 = np.ascontiguousarray(
        np.broadcast_to(np.arange(P, dtype=np.float32)[None, :], (P, P)))
    x = np.asarray(inputs, np.float32)
    in_maps = []
    for r in range(N_CORES):
        pair, h = r // 2, r % 2
        gidx_w, growl_m, gval_m = per_parity[h]
        # xt[b, cc, cl, v] = x[4p+b, h*VH + v, t, f], c=(t,f)=cc*128+cl
        xs = x[BG * pair:BG * (pair + 1), h * VH:(h + 1) * VH]  # [4, VH, T, FIN]
        xt = xs.reshape(BG, VH, C).transpose(0, 2, 1).reshape(BG, 2, P, VH)
        m = {
            "xt": np.ascontiguousarray(xt).astype(ml_dtypes.bfloat16),
            "wz": wz,
            "bias128": bias128,
            "iota128": iota128,
            "offt": np.array([[h * VH]], np.int32),
            "gidx": gidx_w,
            "growl": growl_m,
            "gval1": gval_m,
            "gval2": np.ascontiguousarray(2.0 * gval_m),
        }
        in_maps.append(m)
    return in_maps, chunks


_CACHE = {}


def _get_program(chunks):
    key = tuple(chunks)
    if key not in _CACHE:
        _CACHE[key] = build_program(list(chunks))
    return _CACHE[key]


def kernel(inputs, weight, bias, lap_vals, lap_rows, lap_cols):
    in_maps, chunks = make_host_inputs(inputs, weight, bias, lap_vals,
                                       lap_rows, lap_cols)
    nc = _get_program(chunks)
    res = run_bass_kernel_spmd(nc, in_maps, list(range(N_CORES)))
    out = np.empty((B, V, FOUT), np.float32)
    for r in range(N_CORES):
        pair, h = r // 2, r % 2
        o = res.results[r]["out"].reshape(VH, BG, FOUT)
        out[BG * pair:BG * (pair + 1), h * VH:(h + 1) * VH, :] = \
            o.transpose(1, 0, 2)
    return np.ascontiguousarray(out)


def time_kernel(inputs_dict, iters=3):
    """Wall-clock repeated executions of the cached program (ns per run)."""
    import time

    in_maps, chunks = make_host_inputs(**inputs_dict)
    nc = _get_program(chunks)
    times = []
    for _ in range(iters):
        t0 = time.perf_counter()
        run_bass_kernel_spmd(nc, in_maps, list(range(N_CORES)))
        times.append(time.perf_counter() - t0)
    return min(times) * 1e9


# revision 24
# speedup vs baseline: 1.9380x; 1.0075x over previous
"""Trainium2 Bass kernel for ConvChebTemp (Chebyshev graph conv, temporal weights).

Math: out[b,v,o] = sum_{k,t,f} T_k(L)x0[:,t,f,b] w[f,k,t,o] + bias[o]
with x0 = inputs permuted to [V, T*Fin*B] and T_k the Chebyshev recurrence.

Clenshaw reformulation (weights contracted first):
  z_k[v,b,o] = sum_{t,f} x0[v,t,f,b] w[f,k,t,o]
  b3 = z3; b2 = z2 + 2 L b3; b1 = z1 + 2 L b2 - b3; out = z0 + L b1 - b2 + bias

Sharding: 8 cores = 4 pairs. Pair p owns batches [4p, 4p+4); within the pair
the graph rows are split in half (core 2p: rows [0, V/2), core 2p+1 the rest).
The Clenshaw iterates b3/b2/b1 live in pair-SHARED HBM tensors
(addr_space="Shared": cores (2k, 2k+1) see one physical buffer), so each
core writes only its half and gathers from the full tensor. Cross-core
ordering is a tiny per-pair AllGather barrier before each phase's gathers.

Everything on the SpMM path is bf16: gather rows are 4 batches x 64 Fout x 2B
= 512B (full DMA descriptor efficiency) and all matmuls run at 1 cycle/row.
"""
import sys

sys.path.insert(0, "/opt/trn_rl_repo")

from contextlib import ExitStack  # noqa: E402

import ml_dtypes  # noqa: E402
import numpy as np  # noqa: E402

from concourse import bacc, bass, mybir, tile  # noqa: E402
from concourse.bass_utils import run_bass_kernel_spmd  # noqa: E402

P = 128
N_CORES = 8
FP32 = mybir.dt.float32
BF16 = mybir.dt.bfloat16
I32 = mybir.dt.int32
I16 = mybir.dt.int16

# Problem dims (hardcoded per spec)
B, V, T, FIN = 16, 12288, 4, 64
KV, KT, FOUT = 4, 4, 64
VH = V // 2                # rows per core
NT = VH // P               # out-tiles per core (48)
BG = 4                     # batches per pair
F = BG * FOUT              # spmm row width (256 bf16 = 512B)
C = T * FIN                # z contraction dim (256)
PAIR_GROUPS = [[0, 1], [2, 3], [4, 5], [6, 7]]
CHUNKS_PER_PIECE = 8       # 1024 gather indices per instruction
DMA_SCRATCH = 16384        # SWDGE ring: 1024 descriptors
WGRP = 8                   # out-tiles per batched shared-HBM write


def _preprocess_lap(lap_rows, lap_cols, lap_vals):
    """Split nnz by row-half, sort by local row, pad to a common per-tile
    chunk structure (identical across cores so one SPMD program serves all).

    Returns (per_parity list of (gidx_wrapped, growl, gval), chunks_per_tile).
    """
    halves = []
    counts_h = []
    for h in (0, 1):
        lo, hi = h * VH, (h + 1) * VH
        m = (lap_rows >= lo) & (lap_rows < hi)
        lrows = lap_rows[m] - lo
        order = np.argsort(lrows, kind="stable")
        lrows = lrows[order]
        cols = lap_cols[m][order]
        vals = lap_vals[m][order]
        counts = np.bincount(lrows // P, minlength=NT)
        halves.append((lrows, cols, vals, counts))
        counts_h.append(counts)
    chunks_per_tile = [
        max(1, int(-(-counts_h[0][t] // P)), int(-(-counts_h[1][t] // P)))
        for t in range(NT)
    ]
    nchunk = sum(chunks_per_tile)
    nnzp = nchunk * P
    out = []
    for lrows, cols, vals, counts in halves:
        gidx = np.zeros(nnzp, np.int16)
        growl = np.zeros(nnzp, np.float32)
        gval = np.zeros(nnzp, np.float32)
        starts = np.zeros(NT + 1, np.int64)
        np.cumsum(counts, out=starts[1:])
        pos = 0
        for t in range(NT):
            n = int(counts[t])
            s = int(starts[t])
            gidx[pos:pos + n] = cols[s:s + n]
            growl[pos:pos + n] = (lrows[s:s + n] - t * P).astype(np.float32)
            gval[pos:pos + n] = vals[s:s + n]
            pos += chunks_per_tile[t] * P
        assert pos == nnzp
        gidx_w = np.tile(gidx.reshape(-1, 16).T.copy(), (8, 1))  # [128, nnzp/16]
        growl_m = growl.reshape(nchunk, P).T.copy()
        gval_m = gval.reshape(nchunk, P).T.copy()
        out.append((np.ascontiguousarray(gidx_w),
                    np.ascontiguousarray(growl_m),
                    np.ascontiguousarray(gval_m)))
    return out, chunks_per_tile


def build_program(chunks_per_tile, n_cores=N_CORES):
    nt = NT
    nchunk = sum(chunks_per_tile)
    nnzp = nchunk * P
    nc = bacc.Bacc("TRN2", target_bir_lowering=False, debug=False,
                   num_devices=n_cores, dynamic_dma_scratch_size=DMA_SCRATCH)

    xt_d = nc.dram_tensor("xt", [BG, 2, P, VH], BF16, kind="ExternalInput")
    wz_d = nc.dram_tensor("wz", [P, 2, KV * FOUT], BF16, kind="ExternalInput")
    onesb_d = nc.dram_tensor("onesb", [1, P], BF16, kind="ExternalInput")
    biasw_d = nc.dram_tensor("biasw", [1, KV * FOUT], BF16, kind="ExternalInput")
    iota_d = nc.dram_tensor("iota128", [P, P], BF16, kind="ExternalInput")
    offt_d = nc.dram_tensor("offt", [1, 1], I32, kind="ExternalInput")
    gidx_d = nc.dram_tensor("gidx", [P, nnzp // 16], I16, kind="ExternalInput")
    growl_d = nc.dram_tensor("growl", [P, nchunk], FP32, kind="ExternalInput")
    gval1_d = nc.dram_tensor("gval1", [P, nchunk], FP32, kind="ExternalInput")
    gval2_d = nc.dram_tensor("gval2", [P, nchunk], FP32, kind="ExternalInput")
    out_d = nc.dram_tensor("out", [VH, F], FP32, kind="ExternalOutput")

    # pair-shared Clenshaw iterates (both cores of a pair see one buffer)
    bsh = [nc.dram_tensor(f"bsh{k}", [V, F], BF16, kind="Internal",
                          addr_space="Shared") for k in range(3)]
    bin_d = [nc.dram_tensor(f"bin{k}", [1, 16], BF16, kind="Internal")
             for k in range(3)]
    bout_d = [nc.dram_tensor(f"bout{k}", [2, 16], BF16, kind="Internal")
              for k in range(3)]

    with tile.TileContext(nc) as tc, ExitStack() as ctx:
        const = ctx.enter_context(tc.tile_pool(name="const", bufs=1))
        zres = ctx.enter_context(tc.tile_pool(name="zres", bufs=1))
        xpool = ctx.enter_context(tc.tile_pool(name="x", bufs=2))
        gpool = ctx.enter_context(tc.tile_pool(name="gbuf", bufs=4))
        spool = ctx.enter_context(tc.tile_pool(name="sel", bufs=3))
        opool = ctx.enter_context(tc.tile_pool(name="ostg", bufs=2))
        bpool = ctx.enter_context(tc.tile_pool(name="bounce", bufs=1))
        psz = ctx.enter_context(tc.tile_pool(name="psz", bufs=3, space="PSUM"))
        pss = ctx.enter_context(tc.tile_pool(name="pss", bufs=4, space="PSUM"))

        # constants + metadata resident in SBUF
        iota_sb = const.tile([P, P], BF16, tag="iota")
        nc.sync.dma_start(iota_sb[:], iota_d[:, :])
        ones_sb = const.tile([1, P], BF16, tag="ones")
        nc.sync.dma_start(ones_sb[:], onesb_d[:, :])
        biasw_sb = const.tile([1, KV * FOUT], BF16, tag="biasw")
        nc.sync.dma_start(biasw_sb[:], biasw_d[:, :])
        wz_sb = const.tile([P, 2, KV * FOUT], BF16, tag="wz")
        nc.sync.dma_start(wz_sb[:], wz_d[:, :, :])
        gidx_sb = const.tile([P, nnzp // 16], I16, tag="gidx")
        nc.sync.dma_start(gidx_sb[:], gidx_d[:, :])
        growl_sb = const.tile([P, nchunk], FP32, tag="growl")
        nc.sync.dma_start(growl_sb[:], growl_d[:, :])
        gval1_sb = const.tile([P, nchunk], FP32, tag="gval1")
        nc.sync.dma_start(gval1_sb[:], gval1_d[:, :])
        gval2_sb = const.tile([P, nchunk], FP32, tag="gval2")
        nc.sync.dma_start(gval2_sb[:], gval2_d[:, :])

        # my row offset into the shared [V, F] tensors (0 or VH); loaded on
        # the Activation engine, which issues the symbolic shared writes
        off_reg = nc.scalar.alloc_register("slab_off")
        nc.scalar.reg_load(off_reg, offt_d[0:1, 0:1])
        off = nc.scalar.snap(off_reg, donate=True, min_val=0, max_val=VH)

        # all z_k resident in SBUF: [P, nt, KV, BG, FOUT] bf16 (96KB/partition)
        z_sb = zres.tile([P, nt, KV, BG, FOUT], BF16, tag="z")

        # ---------- phase Z: z_k = x0 @ w_k (+ bias folded into z0) ----------
        VHH = VH // 2
        for b in range(BG):
          for half in range(2):
            v0 = half * VHH
            xb = xpool.tile([P, 2, VHH], BF16, tag="xb")
            nc.sync.dma_start(
                xb[:], xt_d[b, :, :, v0:v0 + VHH].rearrange("c p v -> p c v"))
            for vt in range(half * nt // 2, (half + 1) * nt // 2):
                zps = psz.tile([P, KV * FOUT], FP32, tag="zps")
                for cc in range(2):
                    nc.tensor.matmul(zps[:],
                                     lhsT=xb[:, cc, vt * P - v0:(vt + 1) * P - v0],
                                     rhs=wz_sb[:, cc, :],
                                     start=(cc == 0), stop=False)
                nc.tensor.matmul(zps[:], lhsT=ones_sb[:, :], rhs=biasw_sb[:, :],
                                 start=False, stop=True)
                # PSUM->SBUF cast copies alternate DVE / Activation
                if vt % 2 == 0:
                    nc.vector.tensor_copy(
                        z_sb[:, vt, :, b, :],
                        zps[:].rearrange("p (k o) -> p k o", o=FOUT))
                else:
                    nc.scalar.activation(
                        out=z_sb[:, vt, :, b, :],
                        in_=zps[:].rearrange("p (k o) -> p k o", o=FOUT),
                        func=mybir.ActivationFunctionType.Copy)

        shared_writes = {0: [], 1: [], 2: []}

        def write_half(kidx, kslot, grp):
            """Batched write of WGRP tiles of z-slot kslot to shared bsh[kidx]."""
            g0 = grp * WGRP
            dst = bsh[kidx][bass.ds(off + g0 * P, WGRP * P), :] \
                .rearrange("(t p) f -> p t f", p=P)
            src = z_sb[:, g0:g0 + WGRP, kslot, :, :] \
                .rearrange("p t b o -> p t (b o)")
            w = nc.scalar.dma_start(dst, src)
            shared_writes[kidx].append(w)

        for grp in range(nt // WGRP):
            write_half(0, 3, grp)

        def pair_barrier(k):
            bsb = bpool.tile([1, 16], BF16, tag=f"bsb{k}")
            rd = nc.sync.dma_start(bsb[:], bsh[k][0:1, 0:16])
            # the bounce read must follow ALL my writes to bsh[k], not just
            # the group that happens to overlap row 0
            for w in shared_writes[k]:
                bass._add_dep_helper(rd.ins, w.ins, sync=True,
                                     reason="barrier after all shared writes")
            nc.sync.dma_start(bin_d[k][0:1, :], bsb[:])
            return nc.gpsimd.collective_compute(
                "AllGather", mybir.AluOpType.bypass, PAIR_GROUPS,
                ins=[bin_d[k][0:1, :]], outs=[bout_d[k][:, :]])

        # ---------- spmm phases ----------
        def spmm_phase(src_d, vals_sb, cc_inst, combine):
            state = {"gb": None, "base": 0, "len": 0}

            def ensure_piece(c):
                while state["gb"] is None or c >= state["base"] + state["len"]:
                    base = 0 if state["gb"] is None else state["base"] + state["len"]
                    plen = min(CHUNKS_PER_PIECE, nchunk - base)
                    gb = gpool.tile([P, plen, F], BF16, tag="gb")
                    s0 = base * P
                    nidx = plen * P
                    g = nc.gpsimd.dma_gather(
                        out_ap=gb[:],
                        in_ap=src_d[:, :],
                        idxs_ap=gidx_sb[:, s0 // 16:(s0 + nidx) // 16],
                        num_idxs=nidx,
                        num_idxs_reg=nidx,
                        elem_size=F,
                    )
                    bass._add_dep_helper(g.ins, cc_inst.ins, sync=True,
                                         reason="pair barrier before gather")
                    state.update(gb=gb, base=base, len=plen)
                return state["gb"], state["base"]

            ci = 0
            for tt in range(nt):
                nck = chunks_per_tile[tt]
                ps = pss.tile([P, F], FP32, tag="ps")
                for k in range(nck):
                    col = ci + k
                    gb, base = ensure_piece(col)
                    sT = spool.tile([P, P], BF16, tag="sT")
                    nc.vector.tensor_scalar(
                        out=sT[:], in0=iota_sb[:],
                        scalar1=growl_sb[:, col:col + 1],
                        scalar2=vals_sb[:, col:col + 1],
                        op0=mybir.AluOpType.is_equal,
                        op1=mybir.AluOpType.mult,
                    )
                    nc.tensor.matmul(ps[:], lhsT=sT[:], rhs=gb[:, col - base, :],
                                     start=(k == 0), stop=(k == nck - 1))
                combine(tt, ps)
                ci += nck

        def zslot(vt, k):
            return z_sb[:, vt, k, :, :].rearrange("p b o -> p (b o)")

        def ps3(ps):
            return ps[:].rearrange("p (b o) -> p b o", o=FOUT)

        # phase 1: b2 = z2 + 2 L b3   (result overwrites z2 slot)
        cc0 = pair_barrier(0)

        def combine1(tt, ps):
            nc.vector.tensor_tensor(out=zslot(tt, 2), in0=zslot(tt, 2),
                                    in1=ps[:], op=mybir.AluOpType.add)
            if (tt + 1) % WGRP == 0:
                write_half(1, 2, tt // WGRP)

        spmm_phase(bsh[0], gval2_sb, cc0, combine1)

        # phase 2: b1 = z1 + 2 L b2 - b3   (result overwrites z1 slot)
        cc1 = pair_barrier(1)

        def combine2(tt, ps):
            nc.vector.tensor_tensor(out=zslot(tt, 1), in0=zslot(tt, 1),
                                    in1=ps[:], op=mybir.AluOpType.add)
            nc.vector.tensor_tensor(out=zslot(tt, 1), in0=zslot(tt, 1),
                                    in1=zslot(tt, 3), op=mybir.AluOpType.subtract)
            if (tt + 1) % WGRP == 0:
                write_half(2, 1, tt // WGRP)

        spmm_phase(bsh[1], gval2_sb, cc1, combine2)

        # phase 3: out = z0 + L b1 - b2 + bias
        cc2 = pair_barrier(2)

        def combine3(tt, ps):
            ot = opool.tile([P, F], FP32, tag="ot")
            nc.vector.tensor_tensor(out=ot[:], in0=ps[:], in1=zslot(tt, 2),
                                    op=mybir.AluOpType.subtract)
            nc.vector.tensor_tensor(out=ot[:], in0=ot[:], in1=zslot(tt, 0),
                                    op=mybir.AluOpType.add)
            nc.sync.dma_start(out_d[tt * P:(tt + 1) * P, :], ot[:])

        spmm_phase(bsh[2], gval1_sb, cc2, combine3)

    nc.compile()
    return nc


def make_host_inputs(inputs, weight, bias, lap_vals, lap_rows, lap_cols):
    per_parity, chunks = _preprocess_lap(
        np.asarray(lap_rows), np.asarray(lap_cols),
        np.asarray(lap_vals, np.float32))
    w = np.asarray(weight, np.float32)
    # wz[(t,f) split cc, (k,o)]
    wz = np.transpose(w, (2, 0, 1, 3)).reshape(C, KV * FOUT)
    wz = np.ascontiguousarray(
        wz.reshape(2, P, KV * FOUT).transpose(1, 0, 2)).astype(ml_dtypes.bfloat16)
    biasw = np.zeros((1, KV * FOUT), np.float32)
    biasw[0, :FOUT] = np.asarray(bias, np.float32)
    biasw = biasw.astype(ml_dtypes.bfloat16)
    onesb = np.ones((1, P), ml_dtypes.bfloat16)
    iota128 = np.ascontiguousarray(
        np.broadcast_to(np.arange(P, dtype=np.float32)[None, :],
                        (P, P))).astype(ml_dtypes.bfloat16)
    x = np.asarray(inputs, np.float32)
    in_maps = []
    for r in range(N_CORES):
        pair, h = r // 2, r % 2
        gidx_w, growl_m, gval_m = per_parity[h]
        # xt[b, cc, cl, v] = x[4p+b, h*VH + v, t, f], c=(t,f)=cc*128+cl
        xs = x[BG * pair:BG * (pair + 1), h * VH:(h + 1) * VH]  # [4, VH, T, FIN]
        xt = xs.reshape(BG, VH, C).transpose(0, 2, 1).reshape(BG, 2, P, VH)
        m = {
            "xt": np.ascontiguousarray(xt).astype(ml_dtypes.bfloat16),
            "wz": wz,
            "biasw": biasw,
            "onesb": onesb,
            "iota128": iota128,
            "offt": np.array([[h * VH]], np.int32),
            "gidx": gidx_w,
            "growl": growl_m,
            "gval1": gval_m,
            "gval2": np.ascontiguousarray(2.0 * gval_m),
        }
        in_maps.append(m)
    return in_maps, chunks


_CACHE = {}


def _get_program(chunks):
    key = tuple(chunks)
    if key not in _CACHE:
        _CACHE[key] = build_program(list(chunks))
    return _CACHE[key]


def kernel(inputs, weight, bias, lap_vals, lap_rows, lap_cols):
    in_maps, chunks = make_host_inputs(inputs, weight, bias, lap_vals,
                                       lap_rows, lap_cols)
    nc = _get_program(chunks)
    res = run_bass_kernel_spmd(nc, in_maps, list(range(N_CORES)))
    out = np.empty((B, V, FOUT), np.float32)
    for r in range(N_CORES):
        pair, h = r // 2, r % 2
        o = res.results[r]["out"].reshape(VH, BG, FOUT)
        out[BG * pair:BG * (pair + 1), h * VH:(h + 1) * VH, :] = \
            o.transpose(1, 0, 2)
    return np.ascontiguousarray(out)


def time_kernel(inputs_dict, iters=3):
    """Wall-clock repeated executions of the cached program (ns per run)."""
    import time

    in_maps, chunks = make_host_inputs(**inputs_dict)
    nc = _get_program(chunks)
    times = []
    for _ in range(iters):
        t0 = time.perf_counter()
        run_bass_kernel_spmd(nc, in_maps, list(range(N_CORES)))
        times.append(time.perf_counter() - t0)
    return min(times) * 1e9
